# revision 3
# baseline (speedup 1.0000x reference)
"""Trainium2 Bass kernel: decoder multi-head attention (B=2, Q=K=2048,
D=1024, 16 heads), sharded over 8 NeuronCores by (batch x query-range).

Each core handles one batch's 512-row query slice for all 16 heads:
  - q/k/v projections on the tensor engine (contraction over D on
    partitions, host supplies transposed activations/weights),
  - scores via a K=65 matmul whose extra contraction row folds the
    additive key-padding mask in for free,
  - softmax without max-subtraction (scores are O(1)): one scalar-engine
    Exp pass whose accum_out gives the row sums, then a per-partition
    reciprocal multiply,
  - normalized weights DMA'd out contiguously as (head, q, k); the host
    transposes to the reference (B, Q, K, N) layout,
  - attention output via PE transposes of the normalized weights,
  - output projection (full d_model contraction; per-core rows complete).
"""

import sys
import threading

if "/opt/trn_rl_repo" not in sys.path:
    sys.path.insert(0, "/opt/trn_rl_repo")

import numpy as np

D_MODEL = 1024
N_HEAD = 16
HEAD_DIM = 64
SCALING = HEAD_DIM ** (-0.5)
B = 2
SEQ = 2048
N_CORES = 8
QS = 512  # query rows per core
MASK_NEG = -80.0

_compiled = threading.Lock()
_state = {}


def _build():
    import concourse.bass as bass
    import concourse.mybir as mybir
    import concourse.tile as tile
    from concourse import bacc
    from concourse.masks import make_identity

    f32 = mybir.dt.float32
    AF = mybir.ActivationFunctionType

    nc = bacc.Bacc("TRN2", target_bir_lowering=False, debug=False,
                   num_devices=N_CORES)

    # ---- I/O -----------------------------------------------------------
    qT = nc.dram_tensor("qT", [D_MODEL, QS], f32, kind="ExternalInput")
    kTin = nc.dram_tensor("kTin", [D_MODEL, SEQ], f32, kind="ExternalInput")
    vTin = nc.dram_tensor("vTin", [D_MODEL, SEQ], f32, kind="ExternalInput")
    WqTs = nc.dram_tensor("WqTs", [D_MODEL, D_MODEL], f32, kind="ExternalInput")
    WkT = nc.dram_tensor("WkT", [D_MODEL, D_MODEL], f32, kind="ExternalInput")
    WvT = nc.dram_tensor("WvT", [D_MODEL, D_MODEL], f32, kind="ExternalInput")
    WoT = nc.dram_tensor("WoT", [D_MODEL, D_MODEL], f32, kind="ExternalInput")
    bq_cols = nc.dram_tensor("bq_cols", [128, 8], f32, kind="ExternalInput")
    bk_cols = nc.dram_tensor("bk_cols", [128, 8], f32, kind="ExternalInput")
    bv_row = nc.dram_tensor("bv_row", [1, D_MODEL], f32, kind="ExternalInput")
    bo_row = nc.dram_tensor("bo_row", [1, D_MODEL], f32, kind="ExternalInput")
    maskbias = nc.dram_tensor("maskbias", [1, SEQ], f32, kind="ExternalInput")

    w_out = nc.dram_tensor("w_out", [N_HEAD, QS, SEQ], f32,
                           kind="ExternalOutput")
    out_sl = nc.dram_tensor("out_sl", [QS, D_MODEL], f32,
                            kind="ExternalOutput")

    # per-head [65, 2048] staging of kT (+bias) with the mask-bias row
    kT_ext_dram = nc.dram_tensor("kT_ext_scratch", [N_HEAD, 65, SEQ], f32)

    with tile.TileContext(nc) as tc:
        with tc.tile_pool(name="singles", bufs=1) as singles:
            ident = singles.tile([128, 128], f32)
            make_identity(nc, ident)
            ones1 = singles.tile([1, 128], f32)
            nc.vector.memset(ones1, 1.0)
            bqc = singles.tile([128, 8], f32)
            nc.sync.dma_start(out=bqc, in_=bq_cols[:, :])
            bkc = singles.tile([128, 8], f32)
            nc.sync.dma_start(out=bkc, in_=bk_cols[:, :])
            bvr = singles.tile([1, D_MODEL], f32)
            nc.sync.dma_start(out=bvr, in_=bv_row[:, :])
            bor = singles.tile([1, D_MODEL], f32)
            nc.sync.dma_start(out=bor, in_=bo_row[:, :])
            mbias = singles.tile([1, SEQ], f32)
            nc.sync.dma_start(out=mbias, in_=maskbias[:, :])

            # residents: qT_ext [65, 16*512] (ones row 64), attn_outT
            qT_ext = singles.tile([65, N_HEAD * QS], f32)
            nc.vector.memset(qT_ext[64:65, :], 1.0)
            attnT = singles.tile([128, 8 * QS], f32)

            # ---- P2: q projection -> qT_ext ---------------------------
            with tc.tile_pool(name="p2w", bufs=8) as p2w, \
                 tc.tile_pool(name="p2x", bufs=8) as p2x, \
                 tc.tile_pool(name="p2ps", bufs=2, space="PSUM") as p2ps:
                wq = []
                qx = []
                for kc in range(8):
                    w = p2w.tile([128, D_MODEL], f32, tag="wq")
                    nc.sync.dma_start(out=w, in_=WqTs[kc * 128:(kc + 1) * 128, :])
                    wq.append(w)
                    x = p2x.tile([128, QS], f32, tag="qx")
                    nc.sync.dma_start(out=x, in_=qT[kc * 128:(kc + 1) * 128, :])
                    qx.append(x)
                for dt in range(8):
                    ps = p2ps.tile([128, QS], f32, tag="psq")
                    for kc in range(8):
                        nc.tensor.matmul(ps, wq[kc][:, dt * 128:(dt + 1) * 128],
                                         qx[kc], start=(kc == 0), stop=(kc == 7))
                    for j in range(2):
                        h = 2 * dt + j
                        nc.scalar.activation(
                            qT_ext[0:64, h * QS:(h + 1) * QS],
                            ps[j * 64:(j + 1) * 64, :],
                            AF.Identity,
                            bias=bqc[j * 64:(j + 1) * 64, dt:dt + 1],
                        )

            # ---- P3: k projection -> kT_ext_dram ----------------------
            with tc.tile_pool(name="p3x", bufs=8) as p3x, \
                 tc.tile_pool(name="p3w", bufs=8) as p3w, \
                 tc.tile_pool(name="p3s", bufs=4) as p3s, \
                 tc.tile_pool(name="p3ps", bufs=2, space="PSUM") as p3ps:
                kx = []
                wk = []
                for kc in range(8):
                    x = p3x.tile([128, SEQ], f32, tag="kx")
                    nc.sync.dma_start(out=x, in_=kTin[kc * 128:(kc + 1) * 128, :])
                    kx.append(x)
                    w = p3w.tile([128, D_MODEL], f32, tag="wk")
                    nc.sync.dma_start(out=w, in_=WkT[kc * 128:(kc + 1) * 128, :])
                    wk.append(w)
                for dt in range(8):
                    stages = [p3s.tile([65, SEQ], f32, tag="kst",
                                       name=f"kst{dt}_{jj}")
                              for jj in range(2)]
                    for blk in range(4):
                        ps = p3ps.tile([128, 512], f32, tag="psk")
                        for kc in range(8):
                            nc.tensor.matmul(
                                ps, wk[kc][:, dt * 128:(dt + 1) * 128],
                                kx[kc][:, blk * 512:(blk + 1) * 512],
                                start=(kc == 0), stop=(kc == 7))
                        for j in range(2):
                            nc.scalar.activation(
                                stages[j][0:64, blk * 512:(blk + 1) * 512],
                                ps[j * 64:(j + 1) * 64, :],
                                AF.Identity,
                                bias=bkc[j * 64:(j + 1) * 64, dt:dt + 1],
                            )
                    for j in range(2):
                        nc.vector.tensor_copy(stages[j][64:65, :], mbias)
                        nc.sync.dma_start(out=kT_ext_dram[2 * dt + j],
                                          in_=stages[j])

            # ---- P4/P5/P6 share the resident v buffer -----------------
            with tc.tile_pool(name="vres", bufs=1) as vres:
                v_sb = vres.tile([128, 16 * D_MODEL], f32)

                # P4: v projection (native [kt, d] layout)
                with tc.tile_pool(name="p4w", bufs=8) as p4w, \
                     tc.tile_pool(name="p4x", bufs=3) as p4x, \
                     tc.tile_pool(name="p4ps", bufs=2, space="PSUM") as p4ps:
                    wv = []
                    for kc in range(8):
                        w = p4w.tile([128, D_MODEL], f32, tag="wv")
                        nc.sync.dma_start(out=w, in_=WvT[kc * 128:(kc + 1) * 128, :])
                        wv.append(w)
                    vT_r = vTin.rearrange("(kc p) t -> p kc t", p=128)
                    for tt in range(16):
                        vt = p4x.tile([128, 8, 128], f32, tag="vt")
                        nc.sync.dma_start(
                            out=vt, in_=vT_r[:, :, tt * 128:(tt + 1) * 128])
                        ps = p4ps.tile([128, D_MODEL], f32, tag="psv")
                        for ob in range(2):
                            po = ps[:, ob * 512:(ob + 1) * 512]
                            for kc in range(8):
                                nc.tensor.matmul(
                                    po, vt[:, kc, :],
                                    wv[kc][:, ob * 512:(ob + 1) * 512],
                                    start=(kc == 0), stop=False)
                            nc.tensor.matmul(
                                po, ones1, bvr[0:1, ob * 512:(ob + 1) * 512],
                                start=False, stop=True, skip_group_check=True)
                        nc.vector.tensor_copy(
                            v_sb[:, tt * D_MODEL:(tt + 1) * D_MODEL], ps)

                # P5: attention
                with tc.tile_pool(name="p5k", bufs=2) as p5k, \
                     tc.tile_pool(name="p5e", bufs=6) as p5e, \
                     tc.tile_pool(name="p5s", bufs=8) as p5s, \
                     tc.tile_pool(name="p5t", bufs=3) as p5t, \
                     tc.tile_pool(name="psS", bufs=1, space="PSUM") as psS, \
                     tc.tile_pool(name="psT", bufs=2, space="PSUM") as psT, \
                     tc.tile_pool(name="psA", bufs=2, space="PSUM") as psA:
                    for h in range(16):
                        kte = p5k.tile([65, SEQ], f32, tag="kte")
                        nc.sync.dma_start(out=kte, in_=kT_ext_dram[h])
                        ews = []
                        for i in range(4):
                            pss = psS.tile([128, SEQ], f32, tag="pss")
                            lhs = qT_ext[0:65,
                                         h * QS + i * 128:h * QS + (i + 1) * 128]
                            for blk in range(4):
                                nc.tensor.matmul(
                                    pss[:, blk * 512:(blk + 1) * 512],
                                    lhs, kte[0:65, blk * 512:(blk + 1) * 512],
                                    start=True, stop=True)
                            ew = p5e.tile([128, SEQ], f32, tag="ew")
                            sm = p5s.tile([128, 1], f32, tag="sm")
                            rc = p5s.tile([128, 1], f32, tag="rc")
                            nc.scalar.activation(ew, pss, AF.Exp, accum_out=sm)
                            nc.vector.reciprocal(rc, sm)
                            nc.vector.tensor_scalar_mul(ew, ew, rc)
                            nc.sync.dma_start(
                                out=w_out[h, i * 128:(i + 1) * 128, :], in_=ew)
                            ews.append(ew)
                        pa = psA.tile([64, 512], f32, tag="pa")
                        for j in range(16):
                            pst = psT.tile([128, 512], f32, tag="pst")
                            for i in range(4):
                                nc.tensor.transpose(
                                    pst[:, i * 128:(i + 1) * 128],
                                    ews[i][:, j * 128:(j + 1) * 128], ident)
                            wt = p5t.tile([128, 512], f32, tag="wt")
                            if j % 2 == 0:
                                nc.vector.tensor_copy(wt, pst)
                            else:
                                nc.scalar.copy(wt, pst)
                            nc.tensor.matmul(
                                pa,
                                v_sb[:, j * D_MODEL + h * 64:
                                     j * D_MODEL + (h + 1) * 64],
                                wt, start=(j == 0), stop=(j == 15),
                                skip_group_check=True)
                        dst = attnT[(h % 2) * 64:(h % 2 + 1) * 64,
                                    (h // 2) * QS:(h // 2 + 1) * QS]
                        if h % 2 == 0:
                            nc.vector.tensor_copy(dst, pa)
                        else:
                            nc.scalar.copy(dst, pa)

                # P6: output projection
                with tc.tile_pool(name="p6w", bufs=8) as p6w, \
                     tc.tile_pool(name="p6o", bufs=3) as p6o, \
                     tc.tile_pool(name="p6ps", bufs=2, space="PSUM") as p6ps:
                    wo = []
                    for dtc in range(8):
                        w = p6w.tile([128, D_MODEL], f32, tag="wo")
                        nc.sync.dma_start(out=w,
                                          in_=WoT[dtc * 128:(dtc + 1) * 128, :])
                        wo.append(w)
                    for tt in range(4):
                        for ob in range(2):
                            ps = p6ps.tile([128, 512], f32, tag="psf")
                            for dtc in range(8):
                                nc.tensor.matmul(
                                    ps,
                                    attnT[:, dtc * QS + tt * 128:
                                          dtc * QS + (tt + 1) * 128],
                                    wo[dtc][:, ob * 512:(ob + 1) * 512],
                                    start=(dtc == 0), stop=False)
                            nc.tensor.matmul(
                                ps, ones1, bor[0:1, ob * 512:(ob + 1) * 512],
                                start=False, stop=True, skip_group_check=True)
                            ot = p6o.tile([128, 512], f32, tag="ot")
                            nc.scalar.copy(ot, ps)
                            nc.sync.dma_start(
                                out=out_sl[tt * 128:(tt + 1) * 128,
                                           ob * 512:(ob + 1) * 512],
                                in_=ot)

    nc.compile()
    return nc


def _get_nc():
    with _compiled:
        if "nc" not in _state:
            _state["nc"] = _build()
    return _state["nc"]


def kernel(query, key, value, encoder_attn_mask, Wq, bq, Wk, bk, Wv, bv,
           Wo, bo):
    from concourse.bass_utils import run_bass_kernel_spmd

    f = np.float32
    query = np.ascontiguousarray(np.asarray(query, f))
    key = np.ascontiguousarray(np.asarray(key, f))
    value = np.ascontiguousarray(np.asarray(value, f))
    mask = np.asarray(encoder_attn_mask)
    Wq = np.asarray(Wq, f); bq = np.asarray(bq, f)
    Wk = np.asarray(Wk, f); bk = np.asarray(bk, f)
    Wv = np.asarray(Wv, f); bv = np.asarray(bv, f)
    Wo = np.asarray(Wo, f); bo = np.asarray(bo, f)

    WqTs = np.ascontiguousarray(Wq.T * SCALING)
    WkT = np.ascontiguousarray(Wk.T)
    WvT = np.ascontiguousarray(Wv.T)
    WoT = np.ascontiguousarray(Wo.T)
    bq_cols = np.ascontiguousarray((bq * SCALING).reshape(8, 128).T)
    bk_cols = np.ascontiguousarray(bk.reshape(8, 128).T)
    bv_row = np.ascontiguousarray(bv.reshape(1, D_MODEL))
    bo_row = np.ascontiguousarray(bo.reshape(1, D_MODEL))

    qT_b = [np.ascontiguousarray(query[b].T) for b in range(B)]
    kT_b = [np.ascontiguousarray(key[b].T) for b in range(B)]
    vT_b = [np.ascontiguousarray(value[b].T) for b in range(B)]
    mb_b = [np.where(mask[b], MASK_NEG, 0.0).astype(f).reshape(1, SEQ)
            for b in range(B)]

    in_maps = []
    for c in range(N_CORES):
        b, q0 = c // 4, (c % 4) * QS
        in_maps.append({
            "qT": np.ascontiguousarray(qT_b[b][:, q0:q0 + QS]),
            "kTin": kT_b[b],
            "vTin": vT_b[b],
            "WqTs": WqTs, "WkT": WkT, "WvT": WvT, "WoT": WoT,
            "bq_cols": bq_cols, "bk_cols": bk_cols,
            "bv_row": bv_row, "bo_row": bo_row,
            "maskbias": mb_b[b],
        })

    global _last_in_maps
    _last_in_maps = in_maps
    nc = _get_nc()
    res = run_bass_kernel_spmd(nc, in_maps, core_ids=list(range(N_CORES)))

    out = np.empty((B, SEQ, D_MODEL), f)
    attn_weights = np.empty((B, SEQ, SEQ, N_HEAD), f)
    for c in range(N_CORES):
        b, q0 = c // 4, (c % 4) * QS
        out[b, q0:q0 + QS] = res.results[c]["out_sl"]
        attn_weights[b, q0:q0 + QS] = res.results[c]["w_out"].transpose(1, 2, 0)
    return out, attn_weights


# revision 4
# speedup vs baseline: 1.3108x; 1.3108x over previous
"""Trainium2 Bass kernel: decoder multi-head attention (B=2, Q=K=2048,
D=1024, 16 heads), sharded over 8 NeuronCores by (batch x query-range).

Core c handles batch c//4, query rows (c%4)*512..+512, all 16 heads.
The k/v projections are additionally sharded by key-block across the 4
cores of each batch and exchanged with an in-group AllGather.

Attention per head: scores via a K=65 matmul whose extra contraction row
folds the additive key-padding mask in; softmax without max-subtraction
(scores are O(1)): one scalar-engine Exp pass whose accum_out yields the
row sums, then a per-partition reciprocal multiply. Normalized weights
are written contiguously as (head, q, k) and transposed on-chip (PE
transpose) for the attention-output matmul. The host reassembles the
reference (B, Q, K, N) layout.
"""

import sys
import threading

if "/opt/trn_rl_repo" not in sys.path:
    sys.path.insert(0, "/opt/trn_rl_repo")

import numpy as np

D_MODEL = 1024
N_HEAD = 16
HEAD_DIM = 64
SCALING = HEAD_DIM ** (-0.5)
B = 2
SEQ = 2048
N_CORES = 8
QS = 512          # query rows per core
KS = 512          # key-block per core (k/v projection shard)
MASK_NEG = -80.0

_compiled = threading.Lock()
_state = {}


def _build():
    import concourse.bass as bass
    import concourse.mybir as mybir
    import concourse.tile as tile
    from concourse import bacc
    from concourse.masks import make_identity

    f32 = mybir.dt.float32
    AF = mybir.ActivationFunctionType

    nc = bacc.Bacc("TRN2", target_bir_lowering=False, debug=False,
                   num_devices=N_CORES)

    # ---- I/O -----------------------------------------------------------
    qT = nc.dram_tensor("qT", [D_MODEL, QS], f32, kind="ExternalInput")
    kTin = nc.dram_tensor("kTin", [D_MODEL, KS], f32, kind="ExternalInput")
    vTin = nc.dram_tensor("vTin", [D_MODEL, KS], f32, kind="ExternalInput")
    WqTs = nc.dram_tensor("WqTs", [D_MODEL, D_MODEL], f32, kind="ExternalInput")
    WkT = nc.dram_tensor("WkT", [D_MODEL, D_MODEL], f32, kind="ExternalInput")
    WvT = nc.dram_tensor("WvT", [D_MODEL, D_MODEL], f32, kind="ExternalInput")
    WoT = nc.dram_tensor("WoT", [D_MODEL, D_MODEL], f32, kind="ExternalInput")
    bq_cols = nc.dram_tensor("bq_cols", [128, 8], f32, kind="ExternalInput")
    bk_cols = nc.dram_tensor("bk_cols", [128, 8], f32, kind="ExternalInput")
    bv_row = nc.dram_tensor("bv_row", [1, D_MODEL], f32, kind="ExternalInput")
    bo_row = nc.dram_tensor("bo_row", [1, D_MODEL], f32, kind="ExternalInput")
    # mask bias for this core's key block
    maskb_sl = nc.dram_tensor("maskb_sl", [1, KS], f32, kind="ExternalInput")

    w_out = nc.dram_tensor("w_out", [N_HEAD, QS, SEQ], f32,
                           kind="ExternalOutput")
    out_sl = nc.dram_tensor("out_sl", [QS, D_MODEL], f32,
                            kind="ExternalOutput")

    # collective buffers (DRAM, Local)
    kT_part = nc.dram_tensor("kT_part", [N_HEAD, 65, KS], f32)
    kT_gath = nc.dram_tensor("kT_gath", [4, N_HEAD, 65, KS], f32)
    v_part = nc.dram_tensor("v_part", [KS, D_MODEL], f32)
    v_all = nc.dram_tensor("v_all", [SEQ, D_MODEL], f32)
    GROUPS = [[0, 1, 2, 3], [4, 5, 6, 7]]

    with tile.TileContext(nc) as tc:
        with tc.tile_pool(name="singles", bufs=1) as singles:
            ident = singles.tile([128, 128], f32)
            make_identity(nc, ident)
            ones1 = singles.tile([1, 128], f32)
            nc.vector.memset(ones1, 1.0)
            bqc = singles.tile([128, 8], f32)
            nc.sync.dma_start(out=bqc, in_=bq_cols[:, :])
            bkc = singles.tile([128, 8], f32)
            nc.sync.dma_start(out=bkc, in_=bk_cols[:, :])
            bvr = singles.tile([1, D_MODEL], f32)
            nc.sync.dma_start(out=bvr, in_=bv_row[:, :])
            bor = singles.tile([1, D_MODEL], f32)
            nc.sync.dma_start(out=bor, in_=bo_row[:, :])
            mbias = singles.tile([1, KS], f32)
            nc.sync.dma_start(out=mbias, in_=maskb_sl[:, :])

            qT_ext = singles.tile([65, N_HEAD * QS], f32)
            nc.vector.memset(qT_ext[64:65, :], 1.0)
            attnT = singles.tile([128, 8 * QS], f32)

            # ---- P3: k projection (this core's key block, all heads) --
            with tc.tile_pool(name="p3x", bufs=8) as p3x, \
                 tc.tile_pool(name="p3w", bufs=8) as p3w, \
                 tc.tile_pool(name="p3s", bufs=4) as p3s, \
                 tc.tile_pool(name="p3ps", bufs=2, space="PSUM") as p3ps:
                kx = []
                wk = []
                for kc in range(8):
                    x = p3x.tile([128, KS], f32, tag="kx")
                    nc.sync.dma_start(out=x, in_=kTin[kc * 128:(kc + 1) * 128, :])
                    kx.append(x)
                    w = p3w.tile([128, D_MODEL], f32, tag="wk")
                    nc.sync.dma_start(out=w, in_=WkT[kc * 128:(kc + 1) * 128, :])
                    wk.append(w)
                for dt in range(8):
                    stages = [p3s.tile([65, KS], f32, tag="kst",
                                       name=f"kst{dt}_{jj}")
                              for jj in range(2)]
                    ps = p3ps.tile([128, KS], f32, tag="psk")
                    for kc in range(8):
                        nc.tensor.matmul(ps, wk[kc][:, dt * 128:(dt + 1) * 128],
                                         kx[kc], start=(kc == 0), stop=(kc == 7))
                    for j in range(2):
                        nc.scalar.activation(
                            stages[j][0:64, :], ps[j * 64:(j + 1) * 64, :],
                            AF.Identity,
                            bias=bkc[j * 64:(j + 1) * 64, dt:dt + 1])
                        nc.vector.tensor_copy(stages[j][64:65, :], mbias)
                        nc.sync.dma_start(out=kT_part[2 * dt + j],
                                          in_=stages[j])
                nc.gpsimd.collective_compute(
                    "AllGather", mybir.AluOpType.bypass,
                    replica_groups=GROUPS,
                    ins=[kT_part[:, :, :]],
                    outs=[kT_gath[:, :, :, :]])

            # ---- P4: v projection (this core's key block) -------------
            with tc.tile_pool(name="p4w", bufs=8) as p4w, \
                 tc.tile_pool(name="p4x", bufs=1) as p4x, \
                 tc.tile_pool(name="p4ps", bufs=2, space="PSUM") as p4ps:
                wv = []
                for kc in range(8):
                    w = p4w.tile([128, D_MODEL], f32, tag="wv")
                    nc.sync.dma_start(out=w, in_=WvT[kc * 128:(kc + 1) * 128, :])
                    wv.append(w)
                vt = p4x.tile([128, 8, KS], f32)
                nc.sync.dma_start(
                    out=vt, in_=vTin.rearrange("(kc p) t -> p kc t", p=128))
                for tl in range(4):
                    ps = p4ps.tile([128, D_MODEL], f32, tag="psv")
                    for ob in range(2):
                        po = ps[:, ob * 512:(ob + 1) * 512]
                        for kc in range(8):
                            nc.tensor.matmul(
                                po, vt[:, kc, tl * 128:(tl + 1) * 128],
                                wv[kc][:, ob * 512:(ob + 1) * 512],
                                start=(kc == 0), stop=False)
                        nc.tensor.matmul(
                            po, ones1, bvr[0:1, ob * 512:(ob + 1) * 512],
                            start=False, stop=True, skip_group_check=True)
                    vo = p4x.tile([128, D_MODEL], f32, tag="vo", bufs=2)
                    nc.vector.tensor_copy(vo, ps)
                    nc.sync.dma_start(
                        out=v_part[tl * 128:(tl + 1) * 128, :], in_=vo)
                nc.gpsimd.collective_compute(
                    "AllGather", mybir.AluOpType.bypass,
                    replica_groups=GROUPS,
                    ins=[v_part[:, :]], outs=[v_all[:, :]])

            # ---- P2: q projection -> qT_ext (overlaps the gathers) ----
            with tc.tile_pool(name="p2w", bufs=8) as p2w, \
                 tc.tile_pool(name="p2x", bufs=8) as p2x, \
                 tc.tile_pool(name="p2ps", bufs=2, space="PSUM") as p2ps:
                wq = []
                qx = []
                for kc in range(8):
                    w = p2w.tile([128, D_MODEL], f32, tag="wq")
                    nc.sync.dma_start(out=w, in_=WqTs[kc * 128:(kc + 1) * 128, :])
                    wq.append(w)
                    x = p2x.tile([128, QS], f32, tag="qx")
                    nc.sync.dma_start(out=x, in_=qT[kc * 128:(kc + 1) * 128, :])
                    qx.append(x)
                for dt in range(8):
                    ps = p2ps.tile([128, QS], f32, tag="psq")
                    for kc in range(8):
                        nc.tensor.matmul(ps, wq[kc][:, dt * 128:(dt + 1) * 128],
                                         qx[kc], start=(kc == 0), stop=(kc == 7))
                    for j in range(2):
                        h = 2 * dt + j
                        nc.scalar.activation(
                            qT_ext[0:64, h * QS:(h + 1) * QS],
                            ps[j * 64:(j + 1) * 64, :],
                            AF.Identity,
                            bias=bqc[j * 64:(j + 1) * 64, dt:dt + 1])

            # ---- P5: attention ---------------------------------------
            with tc.tile_pool(name="vres", bufs=1) as vres, \
                 tc.tile_pool(name="p5k", bufs=2) as p5k, \
                 tc.tile_pool(name="p5e", bufs=6) as p5e, \
                 tc.tile_pool(name="p5s", bufs=12) as p5s, \
                 tc.tile_pool(name="p5t", bufs=3) as p5t, \
                 tc.tile_pool(name="psS", bufs=2, space="PSUM") as psS, \
                 tc.tile_pool(name="psT", bufs=2, space="PSUM") as psT, \
                 tc.tile_pool(name="psA", bufs=2, space="PSUM") as psA:
                v_sb = vres.tile([128, 16 * D_MODEL], f32)
                for tt in range(16):
                    nc.sync.dma_start(
                        out=v_sb[:, tt * D_MODEL:(tt + 1) * D_MODEL],
                        in_=v_all[tt * 128:(tt + 1) * 128, :])
                for h in range(16):
                    kte = p5k.tile([65, SEQ], f32, tag="kte")
                    for g in range(4):
                        nc.sync.dma_start(
                            out=kte[:, g * KS:(g + 1) * KS],
                            in_=kT_gath[g, h])
                    ews = []
                    for i in range(4):
                        lhs = qT_ext[0:65,
                                     h * QS + i * 128:h * QS + (i + 1) * 128]
                        ew = p5e.tile([128, SEQ], f32, tag="ew")
                        sms = []
                        for hf in range(2):
                            pss = psS.tile([128, 1024], f32, tag="pss")
                            for b2 in range(2):
                                blk = hf * 2 + b2
                                nc.tensor.matmul(
                                    pss[:, b2 * 512:(b2 + 1) * 512],
                                    lhs, kte[0:65, blk * 512:(blk + 1) * 512],
                                    start=True, stop=True)
                            sm = p5s.tile([128, 1], f32, tag="sm",
                                          name=f"sm{h}_{i}_{hf}")
                            nc.scalar.activation(
                                ew[:, hf * 1024:(hf + 1) * 1024], pss,
                                AF.Exp, accum_out=sm)
                            sms.append(sm)
                        st = p5s.tile([128, 1], f32, tag="st")
                        nc.vector.tensor_add(st, sms[0], sms[1])
                        rc = p5s.tile([128, 1], f32, tag="rc")
                        nc.vector.reciprocal(rc, st)
                        nc.vector.tensor_scalar_mul(ew, ew, rc)
                        nc.sync.dma_start(
                            out=w_out[h, i * 128:(i + 1) * 128, :], in_=ew)
                        ews.append(ew)
                    pa = psA.tile([64, 512], f32, tag="pa")
                    for j in range(16):
                        pst = psT.tile([128, 512], f32, tag="pst")
                        for i in range(4):
                            nc.tensor.transpose(
                                pst[:, i * 128:(i + 1) * 128],
                                ews[i][:, j * 128:(j + 1) * 128], ident)
                        wt = p5t.tile([128, 512], f32, tag="wt")
                        if j % 2 == 0:
                            nc.vector.tensor_copy(wt, pst)
                        else:
                            nc.scalar.copy(wt, pst)
                        nc.tensor.matmul(
                            pa,
                            v_sb[:, j * D_MODEL + h * 64:
                                 j * D_MODEL + (h + 1) * 64],
                            wt, start=(j == 0), stop=(j == 15),
                            skip_group_check=True)
                    dst = attnT[(h % 2) * 64:(h % 2 + 1) * 64,
                                (h // 2) * QS:(h // 2 + 1) * QS]
                    if h % 2 == 0:
                        nc.vector.tensor_copy(dst, pa)
                    else:
                        nc.scalar.copy(dst, pa)

            # ---- P6: output projection -------------------------------
            with tc.tile_pool(name="p6w", bufs=8) as p6w, \
                 tc.tile_pool(name="p6o", bufs=3) as p6o, \
                 tc.tile_pool(name="p6ps", bufs=2, space="PSUM") as p6ps:
                wo = []
                for dtc in range(8):
                    w = p6w.tile([128, D_MODEL], f32, tag="wo")
                    nc.sync.dma_start(out=w,
                                      in_=WoT[dtc * 128:(dtc + 1) * 128, :])
                    wo.append(w)
                for tt in range(4):
                    for ob in range(2):
                        ps = p6ps.tile([128, 512], f32, tag="psf")
                        for dtc in range(8):
                            nc.tensor.matmul(
                                ps,
                                attnT[:, dtc * QS + tt * 128:
                                      dtc * QS + (tt + 1) * 128],
                                wo[dtc][:, ob * 512:(ob + 1) * 512],
                                start=(dtc == 0), stop=False)
                        nc.tensor.matmul(
                            ps, ones1, bor[0:1, ob * 512:(ob + 1) * 512],
                            start=False, stop=True, skip_group_check=True)
                        ot = p6o.tile([128, 512], f32, tag="ot")
                        nc.scalar.copy(ot, ps)
                        nc.sync.dma_start(
                            out=out_sl[tt * 128:(tt + 1) * 128,
                                       ob * 512:(ob + 1) * 512],
                            in_=ot)

    nc.compile()
    return nc


def _get_nc():
    with _compiled:
        if "nc" not in _state:
            _state["nc"] = _build()
    return _state["nc"]


def kernel(query, key, value, encoder_attn_mask, Wq, bq, Wk, bk, Wv, bv,
           Wo, bo):
    from concourse.bass_utils import run_bass_kernel_spmd

    f = np.float32
    query = np.ascontiguousarray(np.asarray(query, f))
    key = np.ascontiguousarray(np.asarray(key, f))
    value = np.ascontiguousarray(np.asarray(value, f))
    mask = np.asarray(encoder_attn_mask)
    Wq = np.asarray(Wq, f); bq = np.asarray(bq, f)
    Wk = np.asarray(Wk, f); bk = np.asarray(bk, f)
    Wv = np.asarray(Wv, f); bv = np.asarray(bv, f)
    Wo = np.asarray(Wo, f); bo = np.asarray(bo, f)

    WqTs = np.ascontiguousarray(Wq.T * SCALING)
    WkT = np.ascontiguousarray(Wk.T)
    WvT = np.ascontiguousarray(Wv.T)
    WoT = np.ascontiguousarray(Wo.T)
    bq_cols = np.ascontiguousarray((bq * SCALING).reshape(8, 128).T)
    bk_cols = np.ascontiguousarray(bk.reshape(8, 128).T)
    bv_row = np.ascontiguousarray(bv.reshape(1, D_MODEL))
    bo_row = np.ascontiguousarray(bo.reshape(1, D_MODEL))

    qT_b = [np.ascontiguousarray(query[b].T) for b in range(B)]
    kT_b = [np.ascontiguousarray(key[b].T) for b in range(B)]
    vT_b = [np.ascontiguousarray(value[b].T) for b in range(B)]
    mb_b = [np.where(mask[b], MASK_NEG, 0.0).astype(f).reshape(1, SEQ)
            for b in range(B)]

    in_maps = []
    for c in range(N_CORES):
        b, loc = c // 4, c % 4
        q0 = loc * QS
        k0 = loc * KS
        in_maps.append({
            "qT": np.ascontiguousarray(qT_b[b][:, q0:q0 + QS]),
            "kTin": np.ascontiguousarray(kT_b[b][:, k0:k0 + KS]),
            "vTin": np.ascontiguousarray(vT_b[b][:, k0:k0 + KS]),
            "WqTs": WqTs, "WkT": WkT, "WvT": WvT, "WoT": WoT,
            "bq_cols": bq_cols, "bk_cols": bk_cols,
            "bv_row": bv_row, "bo_row": bo_row,
            "maskb_sl": np.ascontiguousarray(mb_b[b][:, k0:k0 + KS]),
        })

    global _last_in_maps
    _last_in_maps = in_maps
    nc = _get_nc()
    res = run_bass_kernel_spmd(nc, in_maps, core_ids=list(range(N_CORES)))

    out = np.empty((B, SEQ, D_MODEL), f)
    attn_weights = np.empty((B, SEQ, SEQ, N_HEAD), f)
    for c in range(N_CORES):
        b, q0 = c // 4, (c % 4) * QS
        out[b, q0:q0 + QS] = res.results[c]["out_sl"]
        attn_weights[b, q0:q0 + QS] = res.results[c]["w_out"].transpose(1, 2, 0)
    return out, attn_weights
